# revision 11
# baseline (speedup 1.0000x reference)
"""Mixtral-style GQA attention block, tensor-parallel over 8 NeuronCores.

Sharding: core i owns q heads 4i..4i+3 and kv head i (GQA group == 4, so the
kv head's whole group lives on one core).  w_qkv is column-sharded by head,
w_o is row-sharded; the only collective is an AllGather of the per-core
attention outputs (bf16, 512KB per core per token-quarter).  Each core then
computes a disjoint 512-column slice of the final output, so the host-side
unshard is a pure concatenation.

Schedule (single PE instruction stream, kept dense to avoid HAM re-throttle):
  qkv(s0) qkv(s1) scores(q0) qkv(s2) [pv(q0)+AG0 scores(q1)] qkv(s3)
  [pv(q1)+AG1 scores(q2)] oproj(q0) [pv(q2)+AG2 scores(q3)] oproj(q1)
  [pv(q3)+AG3] oproj(q2) oproj(q3)
The QKV projection is split into 4 token-chunk units; each attention
quarter's exp (scalar engine) runs underneath the next projection unit's
matmuls, so the PE never waits on the activation engine.  The four
quarter-AllGathers overlap attention/o_proj compute.

Softmax denominators: exp strips are accumulated on the vector engine
(bf16), then a single all-ones [128x128] matmul reduces across partitions
and broadcasts the row-sums in one shot; normalization is fused into the
PSUM evacuation multiply.

All matmuls run in bf16 (fp32 PSUM accumulation); softmax runs without
max-subtraction (scores are ~N(0,1) by construction, exp cannot overflow).
"""

import numpy as np
import ml_dtypes
from contextlib import ExitStack

import concourse.bass as bass
import concourse.mybir as mybir
import concourse.tile as tile
from concourse import bacc
from concourse.bass_utils import run_bass_kernel_spmd

P = 128
HID = 4096
D = 128
QH = 4                      # local q heads per core
NB = 6                      # projection M-blocks: q0..q3, k, v
KC = HID // P               # contraction chunks over hidden dim
N_CORES = 8
SCALE = float(D) ** -0.5
NEG = -1.0e30

dt = mybir.dt
bf16 = ml_dtypes.bfloat16

F32 = dt.float32
BF16 = dt.bfloat16


def build_nc(t_len=2048, reps=1):
    S = 512                     # token chunk width (= attention quarter)
    NS = t_len // S             # 4
    WQ = NB * P                 # 768
    WO = QH * P                 # 512
    JC = N_CORES * QH           # o_proj contraction chunks (32)

    nc = bacc.Bacc("TRN2", target_bir_lowering=False, debug=False,
                   num_devices=N_CORES)

    hiddenT = nc.dram_tensor("hiddenT", [HID, t_len], BF16, kind="ExternalInput").ap()
    wqkvT = nc.dram_tensor("wqkvT", [HID, WQ], BF16, kind="ExternalInput").ap()
    woT = nc.dram_tensor("woT", [HID, WO], BF16, kind="ExternalInput").ap()
    cos2 = nc.dram_tensor("cos2", [P, t_len], F32, kind="ExternalInput").ap()
    sin2 = nc.dram_tensor("sin2", [P, t_len], F32, kind="ExternalInput").ap()
    maskd = nc.dram_tensor("maskd", [P, P], F32, kind="ExternalInput").ap()
    outp = nc.dram_tensor("outp", [WO, t_len], F32, kind="ExternalOutput").ap()

    hid_src = hiddenT.rearrange("(c p) t -> p c t", p=P)

    with tile.TileContext(nc) as tc:
        with ExitStack() as whole:
            persist = whole.enter_context(tc.tile_pool(name="persist", bufs=1))
            dram = whole.enter_context(tc.tile_pool(name="dram", bufs=1, space="DRAM"))

            # ---- constants ----
            cos2_sb = persist.tile([P, t_len], F32, tag="cos2")
            sin2_sb = persist.tile([P, t_len], F32, tag="sin2")
            mask_sb = persist.tile([P, P], F32, tag="mask")
            ones128 = persist.tile([P, P], BF16, tag="ones128")
            nc.gpsimd.dma_start(cos2_sb[:], cos2[:])
            nc.gpsimd.dma_start(sin2_sb[:], sin2[:])
            nc.gpsimd.dma_start(mask_sb[:], maskd[:])
            nc.vector.memset(ones128[:], 1.0)

            # ---- persistent activations ----
            qk_sb = [persist.tile([P, t_len], BF16, tag=f"qk{mb}", name=f"qk{mb}")
                     for mb in range(5)]
            v_sb = persist.tile([P, t_len], BF16, tag="v", name="v_sb")

            for rep in range(reps):
                bounce = [dram.tile([WO, S], BF16, tag=f"bounce{rep}_{q}",
                                    name=f"bounce{rep}_{q}")
                          for q in range(NS)]
                gathered = [dram.tile([N_CORES * WO, S], BF16,
                                      tag=f"gathered{rep}_{q}",
                                      name=f"gathered{rep}_{q}",
                                      addr_space="Shared")
                            for q in range(NS)]

                # ---------- attention pools (outlive phase-1 pools) ----------
                att = ExitStack()
                p_pool = att.enter_context(tc.tile_pool(name="pstrips", bufs=34))
                acc_pool = att.enter_context(tc.tile_pool(name="accp", bufs=8))
                st_pool = att.enter_context(
                    tc.tile_pool(name="st_psum", bufs=2, space="PSUM"))
                ops_pool = att.enter_context(
                    tc.tile_pool(name="out_psum", bufs=2, space="PSUM"))
                misc = att.enter_context(tc.tile_pool(name="amisc", bufs=4))

                # ---------- phase-1 pools (closed mid-stream, LIFO inner) ----------
                ph1 = ExitStack()
                wq_pool = ph1.enter_context(tc.tile_pool(name="wq", bufs=1))
                hid_pool = ph1.enter_context(tc.tile_pool(name="hid", bufs=4))
                ps_pool = ph1.enter_context(
                    tc.tile_pool(name="ph1ps", bufs=4, space="PSUM"))
                stage = ph1.enter_context(tc.tile_pool(name="stage", bufs=5))

                # cache all of wqkvT in SBUF: chunk c at [:, c*WQ:(c+1)*WQ]
                wqkv_sb = wq_pool.tile([P, KC * WQ], BF16, tag="wqkv")
                wqkv_src = wqkvT.rearrange("(c p) w -> p c w", p=P)
                wqkv_dst = wqkv_sb.rearrange("p (c w) -> p c w", w=WQ)
                for c0 in range(0, KC, 8):
                    nc.scalar.dma_start(wqkv_dst[:, c0:c0 + 8, :],
                                        wqkv_src[:, c0:c0 + 8, :])

                strips = {}
                accs = {}

                def ph1_s(s):
                    """QKV projection + rope for token chunk s."""
                    ts = slice(s * S, (s + 1) * S)
                    hts = []
                    for gi, cg in enumerate(range(0, KC, 8)):
                        ht = hid_pool.tile([P, 8 * S], BF16, tag="hidstream",
                                           name=f"hid{rep}_{s}_{cg}")
                        hdst = ht.rearrange("p (c t) -> p c t", t=S)
                        nc.sync.dma_start(hdst[:, :, :], hid_src[:, cg:cg + 8, ts])
                        hts.append(ht)
                    for g in range(2):          # mb groups of 3
                        mbs = range(3 * g, 3 * g + 3)
                        pss = [ps_pool.tile([P, S], F32, tag="ph1ps",
                                            name=f"pj{rep}_{s}_{mb}")
                               for mb in mbs]
                        for c in range(KC):
                            for mi, mb in enumerate(mbs):
                                lhsT = wqkv_sb[:, c * WQ + mb * P:c * WQ + (mb + 1) * P]
                                nc.tensor.matmul(
                                    pss[mi][:, :], lhsT=lhsT,
                                    rhs=hts[c // 8][:, (c % 8) * S:(c % 8 + 1) * S],
                                    start=(c == 0), stop=(c == KC - 1))
                        for mi, mb in enumerate(mbs):
                            if mb < 5:
                                # rope: qk[d] = raw[d]*cos2[d] + raw[(d+64)%128]*sin2[d]
                                raw = stage.tile([P, S], F32, tag="stg", name="raw")
                                nc.vector.tensor_copy(raw[:], pss[mi][:])
                                rot = stage.tile([P, S], F32, tag="stg", name="rot")
                                nc.gpsimd.dma_start(rot[0:64, :], raw[64:128, :])
                                nc.gpsimd.dma_start(rot[64:128, :], raw[0:64, :])
                                t1 = stage.tile([P, S], F32, tag="stg", name="t1")
                                nc.vector.tensor_mul(t1[:], raw[:], cos2_sb[:, ts])
                                t2 = stage.tile([P, S], F32, tag="stg", name="t2")
                                nc.vector.tensor_mul(t2[:], rot[:], sin2_sb[:, ts])
                                nc.vector.tensor_add(qk_sb[mb][:, ts], t1[:], t2[:])
                            else:
                                # v: evacuate bf16 [d, t], DMA-transpose each
                                # [d, tk] chunk into [tk, d]
                                vstg = stage.tile([P, S], BF16, tag="vstg",
                                                  name="vstg")
                                nc.vector.tensor_copy(vstg[:], pss[mi][:])
                                for ct in range(S // P):
                                    gc = s * (S // P) + ct
                                    nc.sync.dma_start_transpose(
                                        v_sb[:, gc * P:(gc + 1) * P],
                                        vstg[:, ct * P:(ct + 1) * P])

                def scores_q(q, heads=range(QH)):
                    """Scores + exp + denominator strip-accumulation, quarter q."""
                    tq0 = q * S
                    ncv = (tq0 + S) // P
                    kT = qk_sb[4]
                    for h in heads:
                        qT = qk_sb[h]
                        acc = acc_pool.tile([P, S], BF16, tag="acc",
                                            name=f"acc{rep}_{q}_{h}")
                        lst = []
                        for c in range(ncv):
                            off = max(tq0, P * c)
                            w = tq0 + S - off
                            st = st_pool.tile([P, S], F32, tag="st",
                                              name=f"st{rep}_{q}_{h}_{c}")
                            nc.tensor.matmul(st[:, 0:w], lhsT=kT[:, c * P:(c + 1) * P],
                                             rhs=qT[:, off:off + w],
                                             start=True, stop=True)
                            if P * c >= tq0:
                                # strip starts on the diagonal: mask tq<tk
                                nc.vector.tensor_add(st[:, 0:P], st[:, 0:P],
                                                     mask_sb[:])
                            pt = p_pool.tile([P, S], BF16, tag="p",
                                             name=f"p{rep}_{q}_{h}_{c}")
                            nc.scalar.activation(pt[:, 0:w], st[:, 0:w],
                                                 mybir.ActivationFunctionType.Exp,
                                                 scale=SCALE)
                            b = off - tq0
                            if c == 0:
                                nc.vector.tensor_copy(acc[:, :], pt[:, :])
                            else:
                                nc.vector.tensor_add(acc[:, b:b + w],
                                                     acc[:, b:b + w], pt[:, 0:w])
                            lst.append((pt, off, w))
                        strips[(q, h)] = lst
                        accs[(q, h)] = acc

                def pv_q(q, heads=range(QH), gather=False):
                    """PV matmuls + fused normalization + bounce (+ AllGather)."""
                    tq0 = q * S
                    for h in heads:
                        lst = strips.pop((q, h))
                        acc = accs.pop((q, h))
                        # row-sums broadcast to all partitions in one matmul
                        lbc = st_pool.tile([P, S], F32, tag="st",
                                           name=f"lbc{rep}_{q}_{h}")
                        nc.tensor.matmul(lbc[:, :], lhsT=ones128[:], rhs=acc[:, :],
                                         start=True, stop=True)
                        inv = misc.tile([P, S], F32, tag="inv", name="inv")
                        nc.vector.reciprocal(inv[:], lbc[:])
                        out_ps = ops_pool.tile([P, S], F32, tag="ops",
                                               name=f"ops{rep}_{q}_{h}")
                        cmax = len(lst) - 1
                        for c, (pt, off, w) in enumerate(lst):
                            b = off - tq0
                            nc.tensor.matmul(out_ps[:, b:b + w],
                                             lhsT=v_sb[:, c * P:(c + 1) * P],
                                             rhs=pt[:, 0:w],
                                             start=(c == 0), stop=(c == cmax))
                        outT = misc.tile([P, S], BF16, tag="outT", name="outT")
                        nc.vector.tensor_mul(outT[:], out_ps[:], inv[:])
                        nc.sync.dma_start(bounce[q][h * P:(h + 1) * P, :], outT[:])
                    if gather:
                        nc.gpsimd.collective_compute(
                            "AllGather",
                            mybir.AluOpType.bypass,
                            ins=[bounce[q][:]],
                            outs=[gathered[q][:]],
                            replica_groups=[list(range(N_CORES))],
                        )

                # ---------- emission ----------
                ph1_s(0)
                ph1_s(1)
                scores_q(0)
                ph1_s(2)
                pv_q(0, gather=True)
                scores_q(1)
                ph1_s(3)
                ph1.close()

                # ---------- o_proj pools ----------
                op = ExitStack()
                wo_pool = op.enter_context(tc.tile_pool(name="wo", bufs=1))
                ag_pool = op.enter_context(tc.tile_pool(name="ag", bufs=1))
                po_pool = op.enter_context(
                    tc.tile_pool(name="oproj_psum", bufs=2, space="PSUM"))
                ostg = op.enter_context(tc.tile_pool(name="ostg", bufs=3))

                # cache all of woT in SBUF: chunk c at [:, c*WO:(c+1)*WO]
                wo_sb = wo_pool.tile([P, JC * WO], BF16, tag="wo")
                wo_src = woT.rearrange("(c p) w -> p c w", p=P)
                wo_dst = wo_sb.rearrange("p (c w) -> p c w", w=WO)
                for c0 in range(0, JC, 8):
                    nc.gpsimd.dma_start(wo_dst[:, c0:c0 + 8, :],
                                        wo_src[:, c0:c0 + 8, :])

                ag_tiles = {}

                def oproj_load(q):
                    ag_sb = ag_pool.tile([P, JC * S], BF16, tag="ag",
                                         name=f"ag{rep}_{q}")
                    ag_src = gathered[q].rearrange("(c p) t -> p c t", p=P)
                    ag_dst = ag_sb.rearrange("p (c t) -> p c t", t=S)
                    for cg in range(0, JC, 4):
                        nc.gpsimd.dma_start(ag_dst[:, cg:cg + 4, :],
                                            ag_src[:, cg:cg + 4, :])
                    ag_tiles[q] = ag_sb

                def oproj_pass(q, half):
                    ag_sb = ag_tiles[q]
                    mbs = (2 * half, 2 * half + 1)
                    pos = [po_pool.tile([P, S], F32, tag="po",
                                        name=f"po{rep}_{q}_{mb}")
                           for mb in mbs]
                    for c in range(JC):
                        for mi, mb in enumerate(mbs):
                            lhsT = wo_sb[:, c * WO + mb * P:c * WO + (mb + 1) * P]
                            nc.tensor.matmul(
                                pos[mi][:, :], lhsT=lhsT,
                                rhs=ag_sb[:, c * S:(c + 1) * S],
                                start=(c == 0), stop=(c == JC - 1))
                    for mi, mb in enumerate(mbs):
                        ob = ostg.tile([P, S], F32, tag="ob", name="ob")
                        nc.vector.tensor_copy(ob[:], pos[mi][:])
                        nc.scalar.dma_start(
                            outp[mb * P:(mb + 1) * P, q * S:(q + 1) * S],
                            ob[:])
                    if half == 1:
                        del ag_tiles[q]

                # attention quarters 2/3 interleaved with o_proj half-passes:
                # the o_proj matmul blocks give the scalar engine time to
                # finish the exp strips before the PV matmuls consume them.
                pv_q(1, gather=True)
                scores_q(2, (0, 1))
                oproj_load(0)
                oproj_pass(0, 0)
                pv_q(2, (0, 1))
                scores_q(2, (2, 3))
                oproj_pass(0, 1)
                pv_q(2, (2, 3), gather=True)
                scores_q(3, (0, 1))
                oproj_load(1)
                oproj_pass(1, 0)
                pv_q(3, (0, 1))
                scores_q(3, (2, 3))
                oproj_pass(1, 1)
                pv_q(3, (2, 3), gather=True)
                oproj_load(2)
                oproj_pass(2, 0)
                oproj_pass(2, 1)
                oproj_load(3)
                oproj_pass(3, 0)
                oproj_pass(3, 1)
                op.close()
                att.close()

    nc.compile()
    return nc


def make_inputs(positions, hidden_states, w_qkv, w_o):
    """Host-side shard + relayout.  Returns per-core input maps."""
    half = D // 2
    inv_freq = 1.0 / (1e6 ** (np.arange(0, half, dtype=np.float32) / half))
    freqs = positions.astype(np.float32)[:, None] * inv_freq[None, :]
    cosT = np.cos(freqs).T.astype(np.float32)      # [64, T]
    sinT = np.sin(freqs).T.astype(np.float32)
    cos2 = np.ascontiguousarray(np.concatenate([cosT, cosT], axis=0))
    sin2 = np.ascontiguousarray(np.concatenate([-sinT, sinT], axis=0))

    ii = np.arange(P)
    maskd = np.where(ii[None, :] >= ii[:, None], 0.0, NEG).astype(np.float32)

    hiddenT = np.ascontiguousarray(hidden_states.T).astype(bf16)

    q_size = 32 * D
    in_maps = []
    for i in range(N_CORES):
        rows = np.concatenate([
            w_qkv[QH * P * i:QH * P * (i + 1)],                      # 4 q heads
            w_qkv[q_size + P * i:q_size + P * (i + 1)],              # k head
            w_qkv[q_size + 8 * D + P * i:q_size + 8 * D + P * (i + 1)],  # v head
        ], axis=0)
        wqkvT_i = np.ascontiguousarray(rows.T).astype(bf16)
        woT_i = np.ascontiguousarray(w_o[QH * P * i:QH * P * (i + 1), :].T).astype(bf16)
        in_maps.append({
            "hiddenT": hiddenT,
            "wqkvT": wqkvT_i,
            "woT": woT_i,
            "cos2": cos2,
            "sin2": sin2,
            "maskd": maskd,
        })
    return in_maps


def assemble(results, t_len=2048):
    final = np.empty((t_len, N_CORES * QH * P), dtype=np.float32)
    for i in range(N_CORES):
        final[:, QH * P * i:QH * P * (i + 1)] = results[i]["outp"].T
    return final


def kernel(positions, hidden_states, w_qkv, w_o):
    positions = np.asarray(positions)
    hidden_states = np.asarray(hidden_states, dtype=np.float32)
    w_qkv = np.asarray(w_qkv, dtype=np.float32)
    w_o = np.asarray(w_o, dtype=np.float32)
    t_len = hidden_states.shape[0]

    nc = build_nc(t_len)
    in_maps = make_inputs(positions, hidden_states, w_qkv, w_o)
    res = run_bass_kernel_spmd(nc, in_maps, list(range(N_CORES)))
    return assemble(res.results, t_len)
